# revision 11
# baseline (speedup 1.0000x reference)
"""ErnieLayout self-attention on 8 Trainium2 NeuronCores (Bass/Tile).

Problem shapes (hardcoded): B=4, S=1024, H=768, NH=12, HD=64.
Sharding: core c -> (batch b = c//2, head-half hh = c%2, i.e. 6 heads).
Each core computes attention for its 6 heads of one batch element and
writes the [S, 384] column slice of that batch's output.

Sharding/staging strategy (host side, in make_in_maps):
  The attention_mask zeroes out ~half of the key columns: a masked key k
  gets score FLT_MIN -> softmax prob exactly 0, so neither its rel_pos/
  rel_2d_pos column nor its K/V row can influence the output. The shard
  each core receives therefore contains only the unmasked key columns
  (compacted, padded to a multiple of 128 with rel = -3e4 so the padded
  columns also exp to exactly 0 and drop out of both the numerator and
  the ones-column denominator). Tensors are staged in the layout and
  precision the device kernel consumes them in (the kernel computes in
  fp16 throughout, as the previous version did with on-device casts):
    - relT: rel1^T and rel2^T interleaved [6, KP, 2048] fp16 (k on rows,
      q on columns -> 4KB DMA lines; summed on device)
    - xT [768, S], xkvT [768, KP] (compacted keys), w{q,k,v}T [768, 384]
      all fp16 -> phase 1 needs zero PE transposes and zero DVE casts
  All compute of the reference graph (projections, Q.K^T, +rel1+rel2,
  softmax, P.V) runs on device.

Per-core device algorithm (KP = padded key count, KT = KP/128 tiles),
with work spread over all four compute engines:
  phase 1:  Q^T[j,q]  = sum_i wqT[i,j] xT[i,q]   (fp16 matmul, fp32 PSUM,
            ACT copy w/ bias+0.125 scale -> fp16), K^T likewise from xkvT,
            V_aug[k,(h,d|1)] = sum_i xkvT[i,k] wvT[i,d] + bv, ones col
            appended (-> softmax denominator for free).
  phase 2, per head h, per key tile kt:
            r12[kt] = relT[:, :1024] + relT[:, 1024:]     (Pool, fp16)
            psum[k=128, q=1024] = K^T.T @ Q^T   (single fp16-PSUM matmul;
            single-shot so the f16 write is just output rounding)
            psum += r12[kt]                     (DVE in-place psum add)
            pT = exp(psum) -> fp16              (one ACT op per key tile;
            no mask bias needed: masked keys were never staged; padded
            keys carry rel=-6e4 so exp underflows to exactly 0, matching
            the reference's FLT_MIN replacement semantics)
  per head: ctx^T[d|1, q] += V_aug[kt].T @ pT[kt] over kt (fp32 PSUM);
            ctx^T copied out of PSUM (ACT), back-transposed on the PE
            (fp32, exact), out[q, h*64+d] = ctx[q,d] * recip(ctx[q,64])
            (DVE tensor_scalar with per-partition reciprocal); the
            finalize of head h is emitted inside head h+1's loop so the
            in-order PE stream never stalls on it; the final head's
            finalize streams the output DMA per q-tile.
"""

import os
import sys

import numpy as np

for _p in ("/opt/trn_rl_repo",):
    if _p not in sys.path and os.path.isdir(_p):
        sys.path.append(_p)

import concourse.bass as bass
import concourse.mybir as mybir
import concourse.tile as tile
from concourse import bacc
from concourse.bass_utils import run_bass_kernel_spmd
from concourse.masks import make_identity

F32 = mybir.dt.float32
F16 = mybir.dt.float16
AF = mybir.ActivationFunctionType

P = 128
S = 1024
NH = 6        # heads per core
HD = 64
HIN = 768     # model dim (contraction for projections)
HOUT = NH * HD  # 384, per-core projection width
QT = S // P   # 8 query tiles
VW = HD + 1   # 65: V columns + ones column
PAD = -30000.0  # rel padding; two of these sum to -6e4 -> exp == 0


def _build_kernel_body(tc, aps, ktp):
    import contextlib

    nc = tc.nc
    kp = ktp * P
    xT_ap = aps["xT"]
    xkvT_ap = aps["xkvT"]
    relT_ap = aps["relT"]
    out_ap = aps["out"]

    with contextlib.ExitStack() as ctx:
        const = ctx.enter_context(tc.tile_pool(name="const", bufs=1))

        ident = const.tile([P, P], F16)
        make_identity(nc, ident)
        ident32 = const.tile([P, P], F32)
        nc.vector.tensor_copy(ident32[:], ident[:])
        # preload the ACT Exp table while DMAs stream (the first real exp
        # otherwise eats the 1.5us ACT_TABLE_LOAD on the critical path)
        warm = const.tile([P, 1], F16)
        nc.scalar.activation(warm[:], ident32[:, 0:1], AF.Exp, scale=1.0)

        # long-lived tensors
        qt_pool = ctx.enter_context(tc.tile_pool(name="qT", bufs=3))
        kt_pool = ctx.enter_context(tc.tile_pool(name="kT", bufs=3))
        v_pool = ctx.enter_context(tc.tile_pool(name="v", bufs=ktp))

        qT = [qt_pool.tile([P, S], F16, tag="qT", name=f"qT{i}") for i in range(3)]
        kT = [kt_pool.tile([P, kp], F16, tag="kT", name=f"kT{i}") for i in range(3)]
        v_tiles = [
            v_pool.tile([P, NH, VW], F16, tag="v", name=f"v{i}") for i in range(ktp)
        ]

        # rel stream pools first: their SBUF is disjoint from phase-1 pools,
        # so rel DMA + Pool adds run from t=0 and deep fp16 buffering keeps
        # the DMA queues fed across head boundaries.
        rc_pool = ctx.enter_context(tc.tile_pool(name="rc", bufs=2 * ktp))
        r12_pool = ctx.enter_context(tc.tile_pool(name="r12", bufs=3 * ktp))

        # ---------------- phase 1: load + project --------------------------
        with contextlib.ExitStack() as ph1:
            xt_pool = ph1.enter_context(tc.tile_pool(name="xT", bufs=6))
            xkv_pool = ph1.enter_context(tc.tile_pool(name="xkvT", bufs=6))
            wt_pool = ph1.enter_context(tc.tile_pool(name="wT", bufs=18))
            psum1 = ph1.enter_context(tc.tile_pool(name="psum1", bufs=3, space="PSUM"))

            # DMA priority order: Q-projection inputs (x, wq, biases) first
            # so the PE can start projecting ~8us in; then K/V inputs.
            xT = []
            for t in range(6):
                xt_ = xt_pool.tile([P, S], F16, tag="xT", name=f"xT{t}")
                nc.sync.dma_start(xt_[:], xT_ap[t * P:(t + 1) * P, :])
                xT.append(xt_)
            wT = {}
            for t in range(6):
                wt_ = wt_pool.tile([P, HOUT], F16, tag="wT", name=f"wqT{t}")
                nc.sync.dma_start(wt_[:], aps["wqT"][t * P:(t + 1) * P, :])
                wT[("q", t)] = wt_
            bias_sb = {}
            for wname in ("q", "k"):
                bt = const.tile([P, 3], F32, tag=f"b{wname}")
                nc.sync.dma_start(
                    bt[:], aps[f"b{wname}"].rearrange("(a p) -> p a", p=P)
                )
                if wname == "q":
                    nc.vector.tensor_scalar_mul(bt[:], bt[:], 0.125)
                bias_sb[wname] = bt
            xkvT = []
            for t in range(6):
                xk_ = xkv_pool.tile([P, kp], F16, tag="xkvT", name=f"xkvT{t}")
                nc.sync.dma_start(xk_[:], xkvT_ap[t * P:(t + 1) * P, :])
                xkvT.append(xk_)
            for wname in ("k", "v"):
                for t in range(6):
                    wt_ = wt_pool.tile([P, HOUT], F16, tag="wT",
                                       name=f"w{wname}T{t}")
                    nc.sync.dma_start(wt_[:], aps[f"w{wname}T"][t * P:(t + 1) * P, :])
                    wT[(wname, t)] = wt_
            bv_bc = const.tile([P, NH, HD], F32)
            nc.sync.dma_start(
                bv_bc[:],
                aps["bv"].rearrange("(h d) -> h d", d=HD)[None].to_broadcast(
                    (P, NH, HD)
                ),
            )

            # Q^T: 3 fp16 tiles [128, 1024]
            for jt in range(3):
                for qch in range(2):
                    pp = psum1.tile([P, 512], F32, tag="pj")
                    for hc in range(6):
                        nc.tensor.matmul(
                            pp[:],
                            wT[("q", hc)][:, jt * P:(jt + 1) * P],
                            xT[hc][:, qch * 512:(qch + 1) * 512],
                            start=(hc == 0),
                            stop=(hc == 5),
                        )
                    nc.scalar.activation(
                        qT[jt][:, qch * 512:(qch + 1) * 512],
                        pp[:],
                        AF.Identity,
                        bias=bias_sb["q"][:, jt:jt + 1],
                        scale=0.125,
                    )

            # K^T: 3 fp16 tiles [128, KP]
            kchunks = [(0, min(kp, 512))]
            if kp > 512:
                kchunks.append((512, kp - 512))
            for jt in range(3):
                for (k0, kw) in kchunks:
                    pp = psum1.tile([P, 512], F32, tag="pj", name="pk")[:, :kw]
                    for hc in range(6):
                        nc.tensor.matmul(
                            pp[:],
                            wT[("k", hc)][:, jt * P:(jt + 1) * P],
                            xkvT[hc][:, k0:k0 + kw],
                            start=(hc == 0),
                            stop=(hc == 5),
                        )
                    nc.scalar.activation(
                        kT[jt][:, k0:k0 + kw],
                        pp[:],
                        AF.Identity,
                        bias=bias_sb["k"][:, jt:jt + 1],
                        scale=1.0,
                    )

            # V_aug: KT tiles [128, NH, 65] fp16, ones column appended
            for t in range(ktp):
                pv = psum1.tile([P, 512], F32, tag="pj", name="pv")[:, :HOUT]
                for hc in range(6):
                    nc.tensor.matmul(
                        pv[:],
                        xkvT[hc][:, t * P:(t + 1) * P],
                        wT[("v", hc)][:],
                        start=(hc == 0),
                        stop=(hc == 5),
                    )
                nc.vector.memset(v_tiles[t][:], 1.0)
                nc.vector.tensor_add(
                    v_tiles[t][:, :, 0:HD],
                    pv[:].rearrange("p (h d) -> p h d", d=HD),
                    bv_bc[:],
                )

        # ---------------- phase 2: attention per head ----------------
        out_pool = ctx.enter_context(tc.tile_pool(name="outst", bufs=1))
        out_stage = out_pool.tile([P, QT, HOUT], F32, tag="outst")
        pt_pool = ctx.enter_context(tc.tile_pool(name="pT", bufs=2 * ktp))
        sc_pool = ctx.enter_context(tc.tile_pool(name="sc", bufs=4))
        fin_pool = ctx.enter_context(tc.tile_pool(name="fin", bufs=4))
        spsum = ctx.enter_context(tc.tile_pool(name="spsum", bufs=2, space="PSUM"))
        vpsum = ctx.enter_context(tc.tile_pool(name="vpsum", bufs=3, space="PSUM"))
        fpsum = ctx.enter_context(tc.tile_pool(name="fpsum", bufs=1, space="PSUM"))
        ctt_pool = ctx.enter_context(tc.tile_pool(name="ctt", bufs=4))

        def emit_finalize_half(h, half, ctxT_ps_half):
            """Epilogue for head h, query half `half`: copy ctx^T out of
            PSUM, back-transpose to [q, 65], divide by the denominator
            (ACT scale with the DVE per-partition reciprocal). Deferred one
            head (except the last, which finalizes per half as soon as its
            PV group retires, streaming the output DMA)."""
            t_ = ctt_pool.tile([VW, 512], F32, tag="ctxT_sb",
                               name=f"ctxTs{h}_{half}")
            nc.scalar.copy(t_[:], ctxT_ps_half[:])
            cp = fpsum.tile([P, 512], F32, tag="finp", name=f"ctx{h}_{half}")
            # all PE transposes first, then all reads: avoids the per-slot
            # PE-write/engine-read same-bank ping-pong serialization
            for j in range(4):
                nc.tensor.transpose(
                    cp[:, j * VW:(j + 1) * VW],
                    t_[:, j * P:(j + 1) * P],
                    ident32[:VW, :VW],
                )
            for j in range(4):
                qt = half * 4 + j
                sl = j * VW
                rc = fin_pool.tile([P, 1], F32, tag="recip")
                nc.vector.reciprocal(rc[:], cp[:, sl + HD:sl + HD + 1])
                nc.scalar.activation(
                    out_stage[:, qt, h * HD:(h + 1) * HD],
                    cp[:, sl:sl + HD],
                    AF.Identity,
                    scale=rc[:],
                )
                if h == NH - 1:
                    nc.sync.dma_start(
                        out_ap[qt * P:(qt + 1) * P, :], out_stage[:, qt, :]
                    )

        def emit_finalize(h, ctxT_ps):
            for half in range(2):
                emit_finalize_half(h, half, ctxT_ps[half])

        pending_fin = None
        for h in range(NH):
            # rel strips: [128, 2048] fp16 (rel1T | rel2T); summed on Pool
            r12_strips = []
            for kt in range(ktp):
                rcmb = rc_pool.tile([P, 2 * S], F16, tag="rc")
                nc.sync.dma_start(
                    rcmb[:],
                    relT_ap[h].rearrange("(kt p) q -> p kt q", p=P)[:, kt, :],
                )
                r12 = r12_pool.tile([P, S], F16, tag="r12", name=f"r12_{h}_{kt}")
                eng = nc.vector if kt % 3 == 1 else nc.gpsimd
                eng.tensor_add(r12[:], rcmb[:, :S], rcmb[:, S:])
                r12_strips.append(r12)

            dt, rem = divmod(h, 2)
            d0 = rem * HD
            qTh = qT[dt][d0:d0 + HD, :]
            kTh = kT[dt][d0:d0 + HD, :]

            pT_strips = []
            for kt in range(ktp):
                # scores for all 1024 queries of this key tile: two
                # single-shot matmuls into one 2-bank fp32 PSUM tile
                ps = spsum.tile([P, S], F32, tag="sT")
                for qch in range(2):
                    nc.tensor.matmul(
                        ps[:, qch * 512:(qch + 1) * 512],
                        kTh[:, kt * P:(kt + 1) * P],
                        qTh[:, qch * 512:(qch + 1) * 512],
                        start=True,
                        stop=True,
                    )
                # scores + rel1^T + rel2^T -> fp16 (DVE; frees the PSUM bank)
                sc = sc_pool.tile([P, S], F16, tag="sc", name=f"sc{h}_{kt}")
                nc.vector.tensor_add(sc[:], ps[:], r12_strips[kt][:])
                # exp(scores) -> fp16 probs (no mask: see module doc)
                pT_strip = pt_pool.tile([P, S], F16, tag="pT", name=f"pT{h}_{kt}")
                nc.scalar.activation(pT_strip[:], sc[:], AF.Exp, scale=1.0)
                pT_strips.append(pT_strip)
                if kt == 0 and pending_fin is not None:
                    emit_finalize(*pending_fin)
                    pending_fin = None

            # PV: ctx^T[d|1, q] += V_aug[kt].T @ pT[kt]; row 64 of ctx^T is
            # the softmax denominator (ones column of V_aug).
            ctxT_ps = [
                vpsum.tile([VW, 512], F32, tag="ctxT", name=f"ctxT{h}_{i}")
                for i in range(2)
            ]
            for qch in range(2):
                for kt in range(ktp):
                    nc.tensor.matmul(
                        ctxT_ps[qch][:],
                        v_tiles[kt][:, h, :],
                        pT_strips[kt][:, qch * 512:(qch + 1) * 512],
                        start=(kt == 0),
                        stop=(kt == ktp - 1),
                    )
                if h == NH - 1:
                    emit_finalize_half(h, qch, ctxT_ps[qch])
            if h < NH - 1:
                pending_fin = (h, ctxT_ps)


def build_program(ktp):
    """Build and compile the per-core Bass program for KP = ktp*128 padded
    (compacted) key columns. Returns nc."""
    kp = ktp * P
    nc = bacc.Bacc(
        "TRN2",
        target_bir_lowering=False,
        debug=False,
        num_devices=8,
    )
    aps = {
        "xT": nc.dram_tensor("xT", [HIN, S], F16, kind="ExternalInput").ap(),
        "xkvT": nc.dram_tensor("xkvT", [HIN, kp], F16, kind="ExternalInput").ap(),
        "relT": nc.dram_tensor(
            "relT", [NH, kp, 2 * S], F16, kind="ExternalInput"
        ).ap(),
        "wqT": nc.dram_tensor("wqT", [HIN, HOUT], F16, kind="ExternalInput").ap(),
        "wkT": nc.dram_tensor("wkT", [HIN, HOUT], F16, kind="ExternalInput").ap(),
        "wvT": nc.dram_tensor("wvT", [HIN, HOUT], F16, kind="ExternalInput").ap(),
        "bq": nc.dram_tensor("bq", [HOUT], F32, kind="ExternalInput").ap(),
        "bk": nc.dram_tensor("bk", [HOUT], F32, kind="ExternalInput").ap(),
        "bv": nc.dram_tensor("bv", [HOUT], F32, kind="ExternalInput").ap(),
        "out": nc.dram_tensor("out", [S, HOUT], F32, kind="ExternalOutput").ap(),
    }
    with tile.TileContext(nc) as tc:
        _build_kernel_body(tc, aps, ktp)
    nc.compile()
    return nc


def make_in_maps(inputs, ktp):
    """Shard + stage the full inputs into the 8 per-core input maps.

    Per core c: batch b = c//2, head-half hh = c%2. Key columns are
    compacted to the unmasked set of batch b (padded to ktp*128; padded
    rel = -3e4 so padded keys contribute exactly 0 probability)."""
    kp = ktp * P
    hs = np.asarray(inputs["hidden_states"], np.float32)
    am = np.asarray(inputs["attention_mask"]).astype(np.int32)
    rel1 = np.asarray(inputs["rel_pos"], np.float32)
    rel2 = np.asarray(inputs["rel_2d_pos"], np.float32)
    ws = {k: np.asarray(inputs["W" + k[-1]], np.float32) for k in ("wq", "wk", "wv")}
    bs = {k: np.asarray(inputs["b" + k[-1]], np.float32) for k in ("bq", "bk", "bv")}

    per_batch = []
    for b in range(4):
        idx = np.flatnonzero(am[b, 0, 0] == 0)
        cnt = len(idx)
        assert cnt <= kp
        xT = np.ascontiguousarray(hs[b].T.astype(np.float16))  # [768, S]
        xkvT = np.zeros((HIN, kp), np.float16)
        xkvT[:, :cnt] = hs[b][idx].T
        relT = np.full((12, kp, 2 * S), PAD, np.float16)
        relT[:, :cnt, :S] = rel1[b][:, :, idx].transpose(0, 2, 1)
        relT[:, :cnt, S:] = rel2[b][:, :, idx].transpose(0, 2, 1)
        per_batch.append((xT, xkvT, relT))

    in_maps = []
    for c in range(8):
        b, hh = divmod(c, 2)
        xT, xkvT, relT = per_batch[b]
        csl = slice(hh * HOUT, (hh + 1) * HOUT)
        m = {
            "xT": xT,
            "xkvT": xkvT,
            "relT": relT[hh * NH:(hh + 1) * NH],
        }
        for k in ("wq", "wk", "wv"):
            m[k + "T"] = np.ascontiguousarray(ws[k][csl].T.astype(np.float16))
        for k in ("bq", "bk", "bv"):
            m[k] = np.ascontiguousarray(bs[k][csl])
        in_maps.append(m)
    return in_maps


def gather_output(results):
    out = np.empty((4, S, HIN), np.float32)
    for c in range(8):
        b, hh = divmod(c, 2)
        out[b, :, hh * HOUT:(hh + 1) * HOUT] = results[c]["out"].astype(np.float32)
    return out


_NC_CACHE = {}


def kernel(**inputs):
    am = np.asarray(inputs["attention_mask"]).astype(np.int32)
    max_cnt = max(
        int((am[b, 0, 0] == 0).sum()) for b in range(am.shape[0])
    )
    ktp = max(1, -(-max_cnt // P))  # ceil(max unmasked / 128)
    if ktp not in _NC_CACHE:
        _NC_CACHE[ktp] = build_program(ktp)
    nc = _NC_CACHE[ktp]
    in_maps = make_in_maps(inputs, ktp)
    res = run_bass_kernel_spmd(nc, in_maps, list(range(8)))
    return gather_output(res.results)
